# revision 13
# baseline (speedup 1.0000x reference)
"""Trainium2 Bass kernel for the NodeAttentionLayer (GAT-style) problem.

Math (per reference.py):
    h_t = t_input @ W_t; h_o = o_input @ W_o
    s_t = h_t @ a[:F];  s_o = h_o @ a[F:]
    e[i,j]   = leaky_relu(s_t[i] + s_o[j], 0.2)
    att      = softmax(where(adj>0, e, -9e15), axis=1)
    out      = elu(att @ h_o)

On-device identity: with c = (s_t[i]+s_o[j] > 0), v1 = exp(s_o), v2 =
exp(0.2 s_o), r = exp(0.8 s_t):
    att-numerator @ [h_o|1] = r[i] * (W1 @ M1) + (W2 @ M2)
where W1 = v1*[h_o|1], W2 = v2*[h_o|1], M1 = adj*c, M2 = adj - M1; the
ones column carries the softmax denominator; softmax max-trick cancels.

Engine plan per j-tile [128 j x 1024 i]:
  ACT: c = sigmoid(1e30*(s_t + s_o))       (exact 0/1; ties -> 0.5, benign)
  DVE: M1 = c * adj ; (form A) M2 = adj - M1   (quad-batched [128,4,1024])
  PE : form A: T1 += W1@M1, T2 += W2@M2       (2 streams)
       form B: T1 += W1@M1, T2 += W2@adj + (-W2)@M1  (3 streams, no M2 op)
Form B on a subset of quads balances DVE vs PE. GpSimd is never used for
elementwise work (it shares an SBUF port with DVE and stalls it).

Sharding: rows of t_input/adj (N_t) split across 8 cores; o replicated.
Output computed transposed [F, rows]; host transposes back.
"""

import contextlib
import ctypes
import sys
import types

import ml_dtypes
import numpy as np

import concourse.bass as bass
import concourse.mybir as mybir
import concourse.tile as tile
from concourse.vector_clock import ScopedClock

bf16 = ml_dtypes.bfloat16

# ---------------------------------------------------------------------------
# Environment shims (same as baseline)
# ---------------------------------------------------------------------------

def _patch_tile_drain():
    if getattr(tile.TileContext, "_drain_patch_installed", False):
        return

    def _drain_and_barrier(self, tick_clock, wait_clock):
        nop_inst = self.nc.sync.nop(nofuse=True)
        wait_clock.add_sem_waits(
            nop_inst.ins, ScopedClock({None: tick_clock.global_clock})
        )
        ow = list(nop_inst.ins.sync_info.on_wait) if nop_inst.ins.sync_info else []
        if len(ow) > 1:
            nop_inst.ins.sync_info.on_wait = ow[:1]
            for w in ow[1:]:
                extra = self.nc.sync.nop(nofuse=True)
                if extra.ins.sync_info is None:
                    extra.ins.sync_info = mybir.SyncInfo(on_wait=[w], on_update=[])
                else:
                    extra.ins.sync_info.on_wait = [w]
        self.nc.sync.drain()
        self.nc.all_engine_barrier()
        popped = self.nc._tile_sem_poison_stack.pop()
        assert popped is self._sem_poison
        self.nc.clear_and_free_semaphores(list(self.sems.allocated().values()))
        self.nc.all_engine_barrier()

    tile.TileContext._drain_and_barrier = _drain_and_barrier
    tile.TileContext._drain_patch_installed = True


def _install_ntff_hook():
    if "antenv.axon_hooks" in sys.modules:
        return
    import antenv

    state = {"hook": None}
    mod = types.ModuleType("antenv.axon_hooks")
    mod.set_axon_ntff_profile_hook = lambda h: state.__setitem__("hook", h)
    mod.get_axon_ntff_profile_hook = lambda: state["hook"]
    sys.modules["antenv.axon_hooks"] = mod
    antenv.axon_hooks = mod

    try:
        lib = ctypes.CDLL("/opt/axon/libaxon_pjrt.so")
    except OSError:
        return
    if not hasattr(lib, "axon_start_nrt_profile"):
        return
    lib.axon_start_nrt_profile.argtypes = [
        ctypes.POINTER(ctypes.c_int64),
        ctypes.c_size_t,
    ]
    lib.axon_start_nrt_profile.restype = ctypes.c_int64
    lib.axon_stop_nrt_profile.argtypes = [ctypes.c_char_p]
    lib.axon_stop_nrt_profile.restype = ctypes.c_int64

    @contextlib.contextmanager
    def _ntff_hook(output_dir, device_ids):
        import jax

        jax.devices()
        if device_ids:
            ids = (ctypes.c_int64 * len(device_ids))(*device_ids)
            rc = lib.axon_start_nrt_profile(ids, len(device_ids))
        else:
            rc = lib.axon_start_nrt_profile(None, 0)
        if rc != 0:
            raise RuntimeError(f"axon_start_nrt_profile rc={rc}")
        try:
            yield
        finally:
            n = lib.axon_stop_nrt_profile(str(output_dir).encode())
            print(f"profile: {n} file(s) written to {output_dir}", file=sys.stderr)

    state["hook"] = _ntff_hook


_patch_tile_drain()
_install_ntff_hook()


def _split_multi_waits(nc):
    import bass_rust

    k = 0
    for f in nc.m.functions:
        for blk in f.blocks:
            insts = blk.instructions
            out = []
            changed = False
            for inst in insts:
                si = inst.sync_info
                ow = list(si.on_wait) if si is not None else []
                if len(ow) > 1:
                    for w in ow[:-1]:
                        nop = bass_rust.InstNoOp(
                            name=f"waitsplit-{k}", engine=inst.engine
                        )
                        k += 1
                        nop.sync_info = mybir.SyncInfo(on_wait=[w], on_update=[])
                        out.append(nop)
                    si.on_wait = [ow[-1]]
                    changed = True
                out.append(inst)
            if changed:
                blk.instructions = out


# ---------------------------------------------------------------------------
# Problem constants
# ---------------------------------------------------------------------------
N_T, N_O, F_IN, F_OUT = 8192, 8192, 256, 64
N_CORES = 8
R = N_T // N_CORES            # 1024 t-rows per core
NJ = N_O // 128               # 64 j-tiles
NQ = NJ // 4                  # 16 quads (adj DMA batches of 4 tiles)
KC = F_IN // 128              # 2 contraction chunks
F32 = mybir.dt.float32
BF16 = mybir.dt.bfloat16
AF = mybir.ActivationFunctionType
OP = mybir.AluOpType

# Quads processed in "form B" (3 PE streams, no M2 on DVE)
FORM_B_QUADS = frozenset({0, 1, 2, 3, 5, 6, 8, 9, 11, 14})


def _rep_free(ap, reps):
    """View [P, K] tile as [P, K, reps] via innermost step-0."""
    return bass.AP(tensor=ap.tensor, offset=ap.offset, ap=list(ap.ap) + [[0, reps]])


def build_kernel(split_waits=True):
    nc = bass.Bass("TRN2")

    t_T = nc.dram_tensor("t_T", [F_IN, R], BF16, kind="ExternalInput")
    o_T = nc.dram_tensor("o_T", [F_IN, N_O], BF16, kind="ExternalInput")
    wt_d = nc.dram_tensor("wt_d", [F_IN, F_OUT], BF16, kind="ExternalInput")
    wo_d = nc.dram_tensor("wo_d", [F_IN, F_OUT], BF16, kind="ExternalInput")
    a_d = nc.dram_tensor("a_d", [2 * F_OUT, 1], BF16, kind="ExternalInput")
    adjT = nc.dram_tensor("adjT", [N_O, R], BF16, kind="ExternalInput")
    out_d = nc.dram_tensor("out_d", [F_OUT, R], F32, kind="ExternalOutput")

    st_dram = nc.dram_tensor("st_dram", [1, R], BF16, kind="Internal")
    hoT_dram = nc.dram_tensor("hoT_dram", [80, N_O], BF16, kind="Internal")

    NG = 4                      # o-side groups
    GT = NJ // NG               # 16 j-tiles per group
    GQ = NQ // NG               # 4 quads per group

    with tile.TileContext(nc) as tc, contextlib.ExitStack() as ctx:
        S = ctx.enter_context(tc.tile_pool(name="singles", bufs=1))
        adj_pool = ctx.enter_context(tc.tile_pool(name="adj", bufs=4))
        c_pool = ctx.enter_context(tc.tile_pool(name="cq", bufs=4))
        m1_pool = ctx.enter_context(tc.tile_pool(name="m1q", bufs=4))
        m2_pool = ctx.enter_context(tc.tile_pool(name="m2q", bufs=3))
        o_pool = ctx.enter_context(tc.tile_pool(name="op", bufs=2))
        hoT_pool = ctx.enter_context(tc.tile_pool(name="hp", bufs=2))
        acc = ctx.enter_context(tc.tile_pool(name="acc", bufs=1, space="PSUM"))
        mps = ctx.enter_context(tc.tile_pool(name="mps", bufs=2, space="PSUM"))
        hop = ctx.enter_context(tc.tile_pool(name="hop", bufs=1, space="PSUM"))

        # ------------------------------------------------------------------
        # t-side head
        # ------------------------------------------------------------------
        t_sb = S.tile([128, KC, R], BF16)
        nc.sync.dma_start(
            out=t_sb[:, :, :],
            in_=bass.AP(tensor=t_T, offset=0,
                        ap=[[R, 128], [128 * R, KC], [1, R]]))
        wt_sb = S.tile([128, KC, F_OUT], BF16)
        nc.sync.dma_start(
            out=wt_sb[:, :, :],
            in_=bass.AP(tensor=wt_d, offset=0,
                        ap=[[F_OUT, 128], [128 * F_OUT, KC], [1, F_OUT]]))
        wo_sb = S.tile([128, KC, F_OUT], BF16)
        nc.sync.dma_start(
            out=wo_sb[:, :, :],
            in_=bass.AP(tensor=wo_d, offset=0,
                        ap=[[F_OUT, 128], [128 * F_OUT, KC], [1, F_OUT]]))
        a_t = S.tile([F_OUT, 1], BF16)
        nc.sync.dma_start(out=a_t[:, :], in_=a_d[0:F_OUT, :])
        a_o = S.tile([F_OUT, 1], BF16)
        nc.sync.dma_start(out=a_o[:, :], in_=a_d[F_OUT:2 * F_OUT, :])

        warm_ps = mps.tile([F_OUT, 512], F32, tag="mps")
        for i in range(10):
            nc.tensor.matmul(warm_ps[:, :], wt_sb[:, 0, :],
                             t_sb[:, 0, 0:512], start=True, stop=True)

        ht_sb = S.tile([F_OUT, R], BF16)
        for n in range(R // 512):
            ht_ps = mps.tile([F_OUT, 512], F32, tag="mps")
            for c in range(KC):
                nc.tensor.matmul(ht_ps[:, :], wt_sb[:, c, :],
                                 t_sb[:, c, n * 512:(n + 1) * 512],
                                 start=(c == 0), stop=(c == KC - 1))
            nc.vector.tensor_copy(ht_sb[:, n * 512:(n + 1) * 512], ht_ps[:, :])

        st_b = S.tile([1, R], BF16)
        r_b = S.tile([1, R], BF16)
        for n in range(R // 512):
            st_ps = mps.tile([1, 512], F32, tag="mps")
            nc.tensor.matmul(st_ps[:, :], a_t[:, :],
                             ht_sb[:, n * 512:(n + 1) * 512], start=True, stop=True)
            nc.vector.tensor_copy(st_b[:, n * 512:(n + 1) * 512], st_ps[:, :])
            nc.scalar.activation(r_b[:, n * 512:(n + 1) * 512], st_ps[:, :],
                                 AF.Exp, scale=0.8)
        nc.sync.dma_start(out=st_dram[:, :], in_=st_b[0:1, :])
        st_bcast = S.tile([128, R], BF16)
        nc.sync.dma_start(
            out=st_bcast[:, :],
            in_=bass.AP(tensor=st_dram, offset=0, ap=[[0, 128], [1, R]]),
        )

        # ------------------------------------------------------------------
        # o-side, group-pipelined: h_oT, s_o, transpose, exps
        # ------------------------------------------------------------------
        so30 = S.tile([128, NJ], F32)
        nso_sb = S.tile([128, NJ], F32)
        v1_b = S.tile([128, NJ], BF16)
        v2_b = S.tile([128, NJ], BF16)
        # ho65: [h_o | s_o] per j-tile, col 64 = s_o (arrives via transpose)
        ho_sb = S.tile([128, NJ, 80], BF16)

        for g in range(NG):
            j0 = g * 2048
            o_g = o_pool.tile([128, KC, 2048], BF16, tag="oh", name=f"og{g}")
            nc.scalar.dma_start(
                out=o_g[:, :, :],
                in_=bass.AP(tensor=o_T, offset=j0,
                            ap=[[N_O, 128], [128 * N_O, KC], [1, 2048]]))
            hoT_g = hoT_pool.tile([80, 2048], BF16, tag="hoth",
                                  name=f"hoth{g}")
            nc.vector.memset(hoT_g[F_OUT:80, :], 0.0)
            for half in range(2):
                hps = hop.tile([F_OUT, 2, 512], F32, tag="hop",
                               name=f"hop{g}_{half}")
                for ch in range(2):
                    for c in range(KC):
                        nc.tensor.matmul(
                            hps[:, ch, :], wo_sb[:, c, :],
                            o_g[:, c, (half * 2 + ch) * 512:
                                (half * 2 + ch + 1) * 512],
                            start=(c == 0), stop=(c == KC - 1))
                nc.vector.tensor_copy(
                    hoT_g[0:F_OUT, half * 1024:(half + 1) * 1024],
                    hps[:, :, :])
            for h in range(4):
                so_ps = mps.tile([1, 512], F32, tag="mps")
                nc.tensor.matmul(so_ps[:, :], a_o[:, :],
                                 hoT_g[0:F_OUT, h * 512:(h + 1) * 512],
                                 start=True, stop=True)
                nc.vector.tensor_copy(
                    hoT_g[F_OUT:F_OUT + 1, h * 512:(h + 1) * 512],
                    so_ps[:, :])
            nc.scalar.dma_start(out=hoT_dram[:, j0:j0 + 2048],
                                in_=hoT_g[:, :])
            nc.scalar.dma_start_transpose(
                ho_sb[:, g * GT:(g + 1) * GT, :],
                hoT_dram[:, j0:j0 + 2048])
            gsl = slice(g * GT, (g + 1) * GT)
            so_view = ho_sb[:, gsl, F_OUT]
            nc.vector.tensor_scalar(so30[:, gsl], so_view,
                                    1.0e30, None, OP.mult)
            nc.vector.tensor_scalar(nso_sb[:, gsl], so_view,
                                    -1.0, None, OP.mult)
            nc.scalar.activation(v1_b[:, gsl], so_view, AF.Exp)
            nc.scalar.activation(v2_b[:, gsl], so_view, AF.Exp,
                                 scale=0.2)

        # force the sigmoid table load now (all exps are emitted above)
        sig_dummy = S.tile([1, NJ], BF16)
        nc.scalar.activation(sig_dummy[:, :], v2_b[0:1, :], AF.Sigmoid,
                             scale=1.0e30)

        # ------------------------------------------------------------------
        # Main: per group, W-build then 4 quads
        # ------------------------------------------------------------------
        w1_all = S.tile([128, NJ, F_OUT + 1], BF16)
        w2_all = S.tile([128, NJ, F_OUT + 1], BF16)
        w2n_all = S.tile([128, NJ, F_OUT + 1], BF16)

        t1_acc = [acc.tile([F_OUT + 1, 512], F32, tag=f"t1_{n}", name=f"t1_{n}")
                  for n in range(2)]
        t2_acc = [acc.tile([F_OUT + 1, 512], F32, tag=f"t2_{n}", name=f"t2_{n}")
                  for n in range(2)]
        t1_started = [False, False]
        t2_started = [False, False]

        for g in range(NG):
            gsl = slice(g * GT, (g + 1) * GT)
            nc.vector.tensor_tensor(
                w1_all[:, gsl, 0:F_OUT], ho_sb[:, gsl, 0:F_OUT],
                _rep_free(v1_b[:, gsl], F_OUT), OP.mult)
            nc.vector.tensor_copy(w1_all[:, gsl, F_OUT], v1_b[:, gsl])
            nc.vector.tensor_tensor(
                w2_all[:, gsl, 0:F_OUT], ho_sb[:, gsl, 0:F_OUT],
                _rep_free(v2_b[:, gsl], F_OUT), OP.mult)
            nc.vector.tensor_copy(w2_all[:, gsl, F_OUT], v2_b[:, gsl])
            nc.vector.tensor_scalar(w2n_all[:, gsl, :], w2_all[:, gsl, :],
                                    -1.0, None, OP.mult)

            for q in range(g * GQ, (g + 1) * GQ):
                batch = adj_pool.tile([128, 4, R], BF16, tag="adj",
                                      name=f"adj{q}")
                nc.sync.dma_start(
                    out=batch[:, :, :],
                    in_=bass.AP(tensor=adjT, offset=q * 512 * R,
                                ap=[[R, 128], [128 * R, 4], [1, R]]),
                )
                form_b = q in FORM_B_QUADS
                dve_c = g == 0
                last_q = q == NQ - 1
                for half in range(2):
                    cp = c_pool.tile([128, 2, R], BF16, tag="cq",
                                     name=f"c{q}_{half}")
                    for s2 in range(2):
                        s = half * 2 + s2
                        t = q * 4 + s
                        if dve_c:
                            nc.vector.tensor_scalar(
                                cp[:, s2, :], st_bcast[:, :],
                                nso_sb[:, t:t + 1], None, OP.is_gt)
                        else:
                            nc.scalar.activation(
                                cp[:, s2, :], st_bcast[:, :], AF.Sigmoid,
                                bias=so30[:, t:t + 1], scale=1.0e30)
                    m1p = m1_pool.tile([128, 2, R], BF16, tag="m1q",
                                       name=f"m1{q}_{half}")
                    nc.vector.tensor_tensor(
                        m1p[:, :, :], cp[:, :, :],
                        batch[:, 2 * half:2 * half + 2, :], OP.mult)
                    if not form_b:
                        m2p = m2_pool.tile([128, 2, R], BF16, tag="m2q",
                                           name=f"m2{q}_{half}")
                        nc.vector.tensor_tensor(
                            m2p[:, :, :], batch[:, 2 * half:2 * half + 2, :],
                            m1p[:, :, :], OP.subtract)
                    for s2 in range(2):
                        s = half * 2 + s2
                        t = q * 4 + s
                        last_t = last_q and s == 3
                        for n in range(2):
                            sl = slice(n * 512, (n + 1) * 512)
                            nc.tensor.matmul(t1_acc[n][:, :], w1_all[:, t, :],
                                             m1p[:, s2, sl],
                                             start=not t1_started[n],
                                             stop=last_t)
                            t1_started[n] = True
                        if form_b:
                            for n in range(2):
                                sl = slice(n * 512, (n + 1) * 512)
                                nc.tensor.matmul(t2_acc[n][:, :],
                                                 w2_all[:, t, :],
                                                 batch[:, s, sl],
                                                 start=not t2_started[n],
                                                 stop=False)
                                t2_started[n] = True
                            for n in range(2):
                                sl = slice(n * 512, (n + 1) * 512)
                                nc.tensor.matmul(t2_acc[n][:, :],
                                                 w2n_all[:, t, :],
                                                 m1p[:, s2, sl],
                                                 start=False, stop=last_t)
                        else:
                            for n in range(2):
                                sl = slice(n * 512, (n + 1) * 512)
                                nc.tensor.matmul(t2_acc[n][:, :],
                                                 w2_all[:, t, :],
                                                 m2p[:, s2, sl],
                                                 start=not t2_started[n],
                                                 stop=last_t)
                                t2_started[n] = True

        # ------------------------------------------------------------------
        # Tail: H = r*T1 + T2; out = elu(H[:F]/H[F])
        # ------------------------------------------------------------------
        ones65 = S.tile([1, F_OUT + 1], BF16)
        nc.vector.memset(ones65[:, :], 1.0)

        h_sb = S.tile([F_OUT + 1, R], F32)
        for n in range(2):
            sl = slice(n * 512, (n + 1) * 512)
            rb_ps = mps.tile([F_OUT + 1, 512], F32, tag="mps")
            nc.tensor.matmul(rb_ps[:, :], ones65[:, :], r_b[:, sl],
                             start=True, stop=True)
            nc.vector.tensor_copy(h_sb[:, sl], rb_ps[:, :])
            nc.vector.tensor_tensor(h_sb[:, sl], h_sb[:, sl], t1_acc[n][:, :],
                                    OP.mult)
            nc.vector.tensor_tensor(h_sb[:, sl], h_sb[:, sl], t2_acc[n][:, :],
                                    OP.add)

        zr = S.tile([1, R], F32)
        nc.vector.reciprocal(zr[:, :], h_sb[F_OUT:F_OUT + 1, :])
        zr_b = S.tile([1, R], BF16)
        nc.vector.tensor_copy(zr_b[:, :], zr[:, :])

        ot_sb = S.tile([F_OUT, R], F32)
        for n in range(2):
            sl = slice(n * 512, (n + 1) * 512)
            zb_ps = mps.tile([F_OUT, 512], F32, tag="mps")
            nc.tensor.matmul(zb_ps[:, :], ones65[:, 0:F_OUT], zr_b[:, sl],
                             start=True, stop=True)
            nc.vector.tensor_tensor(ot_sb[:, sl], h_sb[0:F_OUT, sl], zb_ps[:, :],
                                    OP.mult)

        # elu(x) = max(x,0) - 1 + exp(min(x,0))
        mn_sb = S.tile([F_OUT, R], F32)
        nc.vector.tensor_scalar(mn_sb[:, :], ot_sb[:, :], 0.0, None, OP.min)
        nc.scalar.activation(mn_sb[:, :], mn_sb[:, :], AF.Exp)
        nc.vector.tensor_scalar(ot_sb[:, :], ot_sb[:, :], 0.0, -1.0, OP.max, OP.add)
        nc.vector.tensor_tensor(ot_sb[:, :], ot_sb[:, :], mn_sb[:, :], OP.add)
        nc.sync.dma_start(out=out_d[:, :], in_=ot_sb[:, :])

    if split_waits:
        _split_multi_waits(nc)
    return nc


_CACHED = {}


def _get_compiled():
    if "nc" not in _CACHED:
        _CACHED["nc"] = build_kernel()
    return _CACHED["nc"]


def kernel(t_input, o_input, W_t, W_o, a, adj, _trace=False):
    from concourse.bass_utils import run_bass_kernel_spmd

    t_input = np.asarray(t_input, dtype=np.float32)
    o_input = np.asarray(o_input, dtype=np.float32)
    W_t = np.asarray(W_t, dtype=np.float32)
    W_o = np.asarray(W_o, dtype=np.float32)
    a = np.asarray(a, dtype=np.float32)
    adj = np.asarray(adj)

    o_T = np.ascontiguousarray(o_input.T).astype(bf16)
    wt_b = W_t.astype(bf16)
    wo_b = W_o.astype(bf16)
    a_b = a.astype(bf16)
    adj_b = adj.astype(bf16)

    in_maps = []
    for m in range(N_CORES):
        rows = slice(m * R, (m + 1) * R)
        in_maps.append(
            {
                "t_T": np.ascontiguousarray(t_input[rows, :].T).astype(bf16),
                "o_T": o_T,
                "wt_d": wt_b,
                "wo_d": wo_b,
                "a_d": a_b,
                "adjT": np.ascontiguousarray(adj_b[rows, :].T),
            }
        )

    nc = _get_compiled()
    res = run_bass_kernel_spmd(
        nc, in_maps, core_ids=list(range(N_CORES)), trace=_trace
    )
    out = np.empty((N_T, F_OUT), dtype=np.float32)
    for m in range(N_CORES):
        out[m * R:(m + 1) * R, :] = res.results[m]["out_d"].T
    if _trace:
        kernel.last_exec_time_ns = res.exec_time_ns
        kernel.last_results = res
    return out
